# revision 6
# baseline (speedup 1.0000x reference)
"""AGCA (adaptive graph channel attention) distributed Bass kernel for TRN2.

Reference computation (per batch row b):
    y   = mean(x[b], axis=(H,W))                    # [CIN]
    y1  = W1 @ y                                    # [HIDE]
    A1  = softmax(w2 * y1)                          # [HIDE]
    y2  = y1 * A1 + A2.T-contract(y1)               # y1@A2
    y3  = relu(w3 * y2)
    out = sigmoid(W4 @ y3)                          # [OP]

Sharding: pure data-parallel over batch. Each of the 8 cores handles
B/8 = 8 batch rows end-to-end; the tiny params are replicated. No
collectives. The kernel is memory-bound on streaming x (64 MiB/core).

Per-core dataflow:
  - x shard viewed as [BL=8, CT=4, 128, 4096] (batch, channel-tile,
    channel-within-tile, H*W).
  - 16 "supertiles" of [128, 2, 4096] f32 (4 MiB) are DMAd (HWDGE via
    the sync engine, triple-buffered) and sum-reduced along the free
    axis by the vector engine into yT tiles [128c, 8b].
  - The 1/4096 mean scale is folded into W1 on the host.
  - Epilogue: small matmuls on the tensor engine (both y1 [8,128] and
    y1T [128,8] layouts are computed so softmax runs along the free
    axis), exp/sigmoid on the scalar engine, elementwise on vector.
  - Output [8, 512] (batch-major) DMAd out; host concatenates shards.
"""

import numpy as np

import concourse.bass as bass
import concourse.mybir as mybir
from concourse.bass_utils import run_bass_kernel_spmd

F32 = mybir.dt.float32

B, CIN, H, W = 64, 512, 64, 64
HW = H * W          # 4096
NCORES = 8
BL = B // NCORES    # 8 batch rows per core
CT = CIN // 128     # 4 channel tiles
HIDE = 128
OP = 512
NBST = 2            # batch rows per supertile
NGRP = BL // NBST   # 4 supertile groups per channel tile
NST = CT * NGRP     # 16 supertiles
NBUF = 3            # streaming buffers


def build_nc(hw: int = HW, nbuf: int = NBUF):
    nc = bass.Bass()

    x_e = nc.declare_dram_parameter("x", [BL, CT, 128, hw], F32, isOutput=False)
    w1t_e = nc.declare_dram_parameter("w1t", [128, CT, HIDE], F32, isOutput=False)
    a2_e = nc.declare_dram_parameter("a2", [HIDE, HIDE], F32, isOutput=False)
    w4t_e = nc.declare_dram_parameter("w4t", [HIDE, OP], F32, isOutput=False)
    scal_e = nc.declare_dram_parameter("scal", [BL, 2], F32, isOutput=False)
    eye_e = nc.declare_dram_parameter("eye8", [BL, BL], F32, isOutput=False)
    out_e = nc.declare_dram_parameter("out", [BL, OP], F32, isOutput=True)

    Exp = mybir.ActivationFunctionType.Exp
    Sigmoid = mybir.ActivationFunctionType.Sigmoid

    from contextlib import ExitStack

    with ExitStack() as ctx:
        bufs = [
            ctx.enter_context(nc.sbuf_tensor(f"buf{j}", [128, NBST, hw], F32))
            for j in range(nbuf)
        ]
        yt = ctx.enter_context(nc.sbuf_tensor("yt", [128, CT, BL], F32))
        w1ts = ctx.enter_context(nc.sbuf_tensor("w1ts", [128, CT, HIDE], F32))
        a2s = ctx.enter_context(nc.sbuf_tensor("a2s", [HIDE, HIDE], F32))
        w4ts = ctx.enter_context(nc.sbuf_tensor("w4ts", [HIDE, OP], F32))
        scals = ctx.enter_context(nc.sbuf_tensor("scals", [BL, 2], F32))
        eyes = ctx.enter_context(nc.sbuf_tensor("eyes", [BL, BL], F32))

        y1s = ctx.enter_context(nc.sbuf_tensor("y1s", [BL, HIDE], F32))
        y1ts = ctx.enter_context(nc.sbuf_tensor("y1ts", [HIDE, BL], F32))
        es = ctx.enter_context(nc.sbuf_tensor("es", [BL, HIDE], F32))
        ss = ctx.enter_context(nc.sbuf_tensor("ss", [BL, 1], F32))
        rs = ctx.enter_context(nc.sbuf_tensor("rs", [BL, 1], F32))
        a1s = ctx.enter_context(nc.sbuf_tensor("a1s", [BL, HIDE], F32))
        t1s = ctx.enter_context(nc.sbuf_tensor("t1s", [BL, HIDE], F32))
        y2s = ctx.enter_context(nc.sbuf_tensor("y2s", [BL, HIDE], F32))
        y3s = ctx.enter_context(nc.sbuf_tensor("y3s", [BL, HIDE], F32))
        y3ts = ctx.enter_context(nc.sbuf_tensor("y3ts", [HIDE, BL], F32))
        outs = ctx.enter_context(nc.sbuf_tensor("outs", [BL, OP], F32))

        y1_ps = ctx.enter_context(nc.psum_tensor("y1_ps", [BL, HIDE], F32))
        y1t_ps = ctx.enter_context(nc.psum_tensor("y1t_ps", [HIDE, BL], F32))
        p2_ps = ctx.enter_context(nc.psum_tensor("p2_ps", [BL, HIDE], F32))
        y3t_ps = ctx.enter_context(nc.psum_tensor("y3t_ps", [HIDE, BL], F32))
        o_ps = ctx.enter_context(nc.psum_tensor("o_ps", [BL, OP], F32))

        slot_sems = [
            ctx.enter_context(nc.semaphore(f"slot_sem{j}")) for j in range(nbuf)
        ]
        out_sem = ctx.enter_context(nc.semaphore("out_sem"))
        param_sem = ctx.enter_context(nc.semaphore("param_sem"))
        red_sem = ctx.enter_context(nc.semaphore("red_sem"))
        pe_sem = ctx.enter_context(nc.semaphore("pe_sem"))
        act_sem = ctx.enter_context(nc.semaphore("act_sem"))

        def st_idx(i):
            return i // CT, i % CT  # (group, channel tile)

        with nc.Block() as block:

            @block.sync
            def _(sync):
                # Stream all supertiles of x; NBUF-deep buffer rotation.
                for i in range(NST):
                    g, ct = st_idx(i)
                    if i >= nbuf:
                        sync.wait_ge(red_sem, i - nbuf + 1)
                    src = x_e[g * NBST:(g + 1) * NBST, ct, :, :].rearrange(
                        "b p w -> p b w"
                    )
                    sync.dma_start(out=bufs[i % nbuf][:, :, :], in_=src).then_inc(
                        slot_sems[i % nbuf], 16
                    )
                # Output DMA after the sigmoid lands in SBUF.
                sync.wait_ge(act_sem, 5)
                sync.dma_start(out=out_e[:, :], in_=outs[:, :]).then_inc(out_sem, 16)
                sync.wait_ge(out_sem, 16)

            @block.scalar
            def _(scalar):
                # Param loads ride the ACT HWDGE queue so they don't delay
                # the x stream on the sync queue.
                scalar.dma_start(out=w1ts[:, :, :], in_=w1t_e[:, :, :]).then_inc(
                    param_sem, 16
                )
                scalar.dma_start(out=a2s[:, :], in_=a2_e[:, :]).then_inc(param_sem, 16)
                scalar.dma_start(out=w4ts[:, :], in_=w4t_e[:, :]).then_inc(
                    param_sem, 16
                )
                scalar.dma_start(out=scals[:, :], in_=scal_e[:, :]).then_inc(
                    param_sem, 16
                )
                scalar.dma_start(out=eyes[:, :], in_=eye_e[:, :]).then_inc(
                    param_sem, 16
                )
                # Epilogue.
                scalar.wait_ge(pe_sem, 8)
                scalar.copy(y1s[:, :], y1_ps[:, :]).then_inc(act_sem, 1)
                scalar.copy(y1ts[:, :], y1t_ps[:, :]).then_inc(act_sem, 1)
                scalar.wait_ge(param_sem, 80)
                scalar.wait_ge(act_sem, 2)  # y1s write retired (same-engine)
                scalar.activation(
                    es[:, :], y1s[:, :], Exp, scale=scals[:, 0:1]
                ).then_inc(act_sem, 1)
                scalar.wait_ge(pe_sem, 10)
                scalar.copy(y3ts[:, :], y3t_ps[:, :]).then_inc(act_sem, 1)
                scalar.wait_ge(pe_sem, 11)
                scalar.activation(outs[:, :], o_ps[:, :], Sigmoid).then_inc(act_sem, 1)

            @block.vector
            def _(vector):
                for i in range(NST):
                    g, ct = st_idx(i)
                    vector.wait_ge(slot_sems[i % nbuf], 16 * (i // nbuf + 1))
                    vector.reduce_sum(
                        yt[:, ct, g * NBST:(g + 1) * NBST],
                        bufs[i % nbuf][:, :, :],
                        axis=mybir.AxisListType.X,
                    ).then_inc(red_sem, 1)
                # Epilogue.
                vector.wait_ge(act_sem, 3)
                vector.reduce_sum(
                    ss[:, :], es[:, :], axis=mybir.AxisListType.X
                ).then_inc(red_sem, 1)
                vector.wait_ge(red_sem, NST + 1)
                vector.reciprocal(rs[:, :], ss[:, :]).then_inc(red_sem, 1)
                vector.wait_ge(red_sem, NST + 2)
                vector.tensor_scalar_mul(a1s[:, :], es[:, :], rs[:, 0:1]).then_inc(
                    red_sem, 1
                )
                vector.wait_ge(red_sem, NST + 3)
                vector.tensor_mul(t1s[:, :], y1s[:, :], a1s[:, :]).then_inc(red_sem, 1)
                vector.wait_ge(pe_sem, 9)
                vector.wait_ge(red_sem, NST + 4)
                vector.tensor_add(y2s[:, :], t1s[:, :], p2_ps[:, :]).then_inc(
                    red_sem, 1
                )
                vector.wait_ge(red_sem, NST + 5)
                vector.tensor_scalar(
                    y3s[:, :],
                    y2s[:, :],
                    scals[:, 1:2],
                    0.0,
                    op0=mybir.AluOpType.mult,
                    op1=mybir.AluOpType.max,
                ).then_inc(red_sem, 1)

            @block.tensor
            def _(tensor):
                tensor.wait_ge(param_sem, 80)
                tensor.wait_ge(red_sem, NST)
                # y1T[h, b] += sum_c W1T[c, h] * yT[c, b]
                for ct in range(CT):
                    tensor.matmul(
                        y1t_ps[:, :],
                        w1ts[:, ct, :],
                        yt[:, ct, :],
                        start=(ct == 0),
                        stop=(ct == CT - 1),
                    ).then_inc(pe_sem, 1)
                # y1[b, h] += sum_c yT[c, b] * W1T[c, h]
                for ct in range(CT):
                    tensor.matmul(
                        y1_ps[:, :],
                        yt[:, ct, :],
                        w1ts[:, ct, :],
                        start=(ct == 0),
                        stop=(ct == CT - 1),
                    ).then_inc(pe_sem, 1)
                # p2[b, k] = sum_h y1T[h, b] * A2[h, k]
                tensor.wait_ge(act_sem, 2)
                tensor.matmul(
                    p2_ps[:, :], y1ts[:, :], a2s[:, :], start=True, stop=True
                ).then_inc(pe_sem, 1)
                # y3T = transpose(y3)
                tensor.wait_ge(red_sem, NST + 6)
                tensor.transpose(y3t_ps[:, :], y3s[:, :], eyes[:, :]).then_inc(
                    pe_sem, 1
                )
                # out[b, o] = sum_h y3T[h, b] * W4T[h, o]
                tensor.wait_ge(act_sem, 4)
                tensor.matmul(
                    o_ps[:, :], y3ts[:, :], w4ts[:, :], start=True, stop=True
                ).then_inc(pe_sem, 1)

    return nc


def prep_in_maps(x, W1, A2, w2, w3, W4, hw: int = HW):
    """Shard x over batch; replicate (pre-transposed) params."""
    x = np.ascontiguousarray(np.asarray(x, dtype=np.float32))
    # W1T with the mean scale folded in: [c, h] -> [128, CT, HIDE] with
    # w1t[p, ct, h] = W1[h, ct*128+p] / hw
    w1t = np.ascontiguousarray(
        (np.asarray(W1, np.float32).T / hw).reshape(CT, 128, HIDE).transpose(1, 0, 2)
    )
    a2 = np.ascontiguousarray(np.asarray(A2, np.float32))
    w4t = np.ascontiguousarray(np.asarray(W4, np.float32).T)
    scal = np.empty((BL, 2), np.float32)
    scal[:, 0] = np.float32(w2)
    scal[:, 1] = np.float32(w3)
    eye8 = np.eye(BL, dtype=np.float32)

    in_maps = []
    for c in range(NCORES):
        xs = x[c * BL:(c + 1) * BL].reshape(BL, CT, 128, hw)
        in_maps.append(
            {
                "x": xs,
                "w1t": w1t,
                "a2": a2,
                "w4t": w4t,
                "scal": scal,
                "eye8": eye8,
            }
        )
    return in_maps


def run(inputs: dict, trace: bool = False):
    """Build + run on 8 cores. Returns (full_output, BassKernelResults)."""
    nc = build_nc()
    in_maps = prep_in_maps(
        inputs["x"], inputs["W1"], inputs["A2"], inputs["w2"], inputs["w3"],
        inputs["W4"],
    )
    res = run_bass_kernel_spmd(
        nc, in_maps, core_ids=list(range(NCORES)), trace=trace
    )
    out = np.concatenate([res.results[c]["out"] for c in range(NCORES)], axis=0)
    return out.reshape(B, OP, 1, 1).astype(np.float32), res


def kernel(**inputs) -> np.ndarray:
    out, _ = run(inputs, trace=False)
    return out


# revision 7
# speedup vs baseline: 11.6863x; 11.6863x over previous
"""AGCA (adaptive graph channel attention) distributed Bass kernel for TRN2.

Reference computation (per batch row b):
    y   = mean(x[b], axis=(H,W))                    # [CIN]
    y1  = W1 @ y                                    # [HIDE]
    A1  = softmax(w2 * y1)                          # [HIDE]
    y2  = y1 * A1 + A2.T-contract(y1)               # y1@A2
    y3  = relu(w3 * y2)
    out = sigmoid(W4 @ y3)                          # [OP]

Sharding: pure data-parallel over batch. Each of the 8 cores handles
B/8 = 8 batch rows end-to-end; the tiny params are replicated. No
collectives. The kernel is memory-bound on streaming x (64 MiB/core).

Per-core dataflow:
  - x shard viewed as [BL=8, CT=4, 128, 4096] (batch, channel-tile,
    channel-within-tile, H*W).
  - 16 "supertiles" of [128, 2, 4096] f32 (4 MiB) are DMAd (HWDGE via
    the sync engine, triple-buffered) and sum-reduced along the free
    axis by the vector engine into yT tiles [128c, 8b].
  - The 1/4096 mean scale is folded into W1 on the host.
  - Epilogue: small matmuls on the tensor engine (both y1 [8,128] and
    y1T [128,8] layouts are computed so softmax runs along the free
    axis), exp/sigmoid on the scalar engine, elementwise on vector.
  - Output [8, 512] (batch-major) DMAd out; host concatenates shards.
"""

import numpy as np

import concourse.bass as bass
import concourse.mybir as mybir
from concourse.bass_utils import run_bass_kernel_spmd

F32 = mybir.dt.float32

B, CIN, H, W = 64, 512, 64, 64
HW = H * W          # 4096
NCORES = 8
BL = B // NCORES    # 8 batch rows per core
CT = CIN // 128     # 4 channel tiles
HIDE = 128
OP = 512
NBST = 2            # batch rows per supertile
NGRP = BL // NBST   # 4 supertile groups per channel tile
NST = CT * NGRP     # 16 supertiles
NBUF = 3            # streaming buffers


def build_nc(hw: int = HW, nbuf: int = NBUF):
    nc = bass.Bass()

    x_e = nc.declare_dram_parameter("x", [BL, CT, 128, hw], F32, isOutput=False)
    w1t_e = nc.declare_dram_parameter("w1t", [128, CT, HIDE], F32, isOutput=False)
    a2_e = nc.declare_dram_parameter("a2", [HIDE, HIDE], F32, isOutput=False)
    w4t_e = nc.declare_dram_parameter("w4t", [HIDE, OP], F32, isOutput=False)
    scal_e = nc.declare_dram_parameter("scal", [BL, 2], F32, isOutput=False)
    eye_e = nc.declare_dram_parameter("eye8", [BL, BL], F32, isOutput=False)
    out_e = nc.declare_dram_parameter("out", [BL, OP], F32, isOutput=True)

    Exp = mybir.ActivationFunctionType.Exp
    Sigmoid = mybir.ActivationFunctionType.Sigmoid

    from contextlib import ExitStack

    with ExitStack() as ctx:
        bufs = [
            ctx.enter_context(nc.sbuf_tensor(f"buf{j}", [128, NBST, hw], F32))
            for j in range(nbuf)
        ]
        yt = ctx.enter_context(nc.sbuf_tensor("yt", [128, CT, BL], F32))
        w1ts = ctx.enter_context(nc.sbuf_tensor("w1ts", [128, CT, HIDE], F32))
        a2s = ctx.enter_context(nc.sbuf_tensor("a2s", [HIDE, HIDE], F32))
        w4ts = ctx.enter_context(nc.sbuf_tensor("w4ts", [HIDE, OP], F32))
        scals = ctx.enter_context(nc.sbuf_tensor("scals", [BL, 2], F32))
        eyes = ctx.enter_context(nc.sbuf_tensor("eyes", [BL, BL], F32))

        y1s = ctx.enter_context(nc.sbuf_tensor("y1s", [BL, HIDE], F32))
        y1ts = ctx.enter_context(nc.sbuf_tensor("y1ts", [HIDE, BL], F32))
        es = ctx.enter_context(nc.sbuf_tensor("es", [BL, HIDE], F32))
        ss = ctx.enter_context(nc.sbuf_tensor("ss", [BL, 1], F32))
        rs = ctx.enter_context(nc.sbuf_tensor("rs", [BL, 1], F32))
        a1s = ctx.enter_context(nc.sbuf_tensor("a1s", [BL, HIDE], F32))
        t1s = ctx.enter_context(nc.sbuf_tensor("t1s", [BL, HIDE], F32))
        y2s = ctx.enter_context(nc.sbuf_tensor("y2s", [BL, HIDE], F32))
        y3s = ctx.enter_context(nc.sbuf_tensor("y3s", [BL, HIDE], F32))
        y3ts = ctx.enter_context(nc.sbuf_tensor("y3ts", [HIDE, BL], F32))
        outs = ctx.enter_context(nc.sbuf_tensor("outs", [BL, OP], F32))

        y1_ps = ctx.enter_context(nc.psum_tensor("y1_ps", [BL, HIDE], F32))
        y1t_ps = ctx.enter_context(nc.psum_tensor("y1t_ps", [HIDE, BL], F32))
        p2_ps = ctx.enter_context(nc.psum_tensor("p2_ps", [BL, HIDE], F32))
        y3t_ps = ctx.enter_context(nc.psum_tensor("y3t_ps", [HIDE, BL], F32))
        o_ps = ctx.enter_context(nc.psum_tensor("o_ps", [BL, OP], F32))

        slot_sems = [
            ctx.enter_context(nc.semaphore(f"slot_sem{j}")) for j in range(nbuf)
        ]
        out_sem = ctx.enter_context(nc.semaphore("out_sem"))
        param_sem = ctx.enter_context(nc.semaphore("param_sem"))
        red_sem = ctx.enter_context(nc.semaphore("red_sem"))
        pe_sem = ctx.enter_context(nc.semaphore("pe_sem"))
        act_sem = ctx.enter_context(nc.semaphore("act_sem"))

        def st_idx(i):
            return i // CT, i % CT  # (group, channel tile)

        with nc.Block() as block:

            @block.sync
            def _(sync):
                # Stream all supertiles of x; NBUF-deep buffer rotation.
                for i in range(NST):
                    g, ct = st_idx(i)
                    if i >= nbuf:
                        sync.wait_ge(red_sem, i - nbuf + 1)
                    src = x_e[g * NBST:(g + 1) * NBST, ct, :, :].rearrange(
                        "b p w -> p b w"
                    )
                    sync.dma_start(out=bufs[i % nbuf][:, :, :], in_=src).then_inc(
                        slot_sems[i % nbuf], 16
                    )
                # Output DMA after the sigmoid lands in SBUF.
                sync.wait_ge(act_sem, 5)
                sync.dma_start(out=out_e[:, :], in_=outs[:, :]).then_inc(out_sem, 16)
                sync.wait_ge(out_sem, 16)

            @block.scalar
            def _(scalar):
                # Param loads ride the ACT HWDGE queue so they don't delay
                # the x stream on the sync queue.
                scalar.dma_start(out=w1ts[:, :, :], in_=w1t_e[:, :, :]).then_inc(
                    param_sem, 16
                )
                scalar.dma_start(out=a2s[:, :], in_=a2_e[:, :]).then_inc(param_sem, 16)
                scalar.dma_start(out=w4ts[:, :], in_=w4t_e[:, :]).then_inc(
                    param_sem, 16
                )
                scalar.dma_start(out=scals[:, :], in_=scal_e[:, :]).then_inc(
                    param_sem, 16
                )
                scalar.dma_start(out=eyes[:, :], in_=eye_e[:, :]).then_inc(
                    param_sem, 16
                )
                # Epilogue.
                scalar.wait_ge(pe_sem, 8)
                scalar.copy(y1s[:, :], y1_ps[:, :]).then_inc(act_sem, 1)
                scalar.copy(y1ts[:, :], y1t_ps[:, :]).then_inc(act_sem, 1)
                scalar.wait_ge(param_sem, 80)
                scalar.wait_ge(act_sem, 2)  # y1s write retired (same-engine)
                scalar.activation(
                    es[:, :], y1s[:, :], Exp, scale=scals[:, 0:1]
                ).then_inc(act_sem, 1)
                scalar.wait_ge(pe_sem, 10)
                scalar.copy(y3ts[:, :], y3t_ps[:, :]).then_inc(act_sem, 1)
                scalar.wait_ge(pe_sem, 11)
                scalar.activation(outs[:, :], o_ps[:, :], Sigmoid).then_inc(act_sem, 1)

            @block.vector
            def _(vector):
                for i in range(NST):
                    g, ct = st_idx(i)
                    vector.wait_ge(slot_sems[i % nbuf], 16 * (i // nbuf + 1))
                    vector.reduce_sum(
                        yt[:, ct, g * NBST:(g + 1) * NBST],
                        bufs[i % nbuf][:, :, :],
                        axis=mybir.AxisListType.X,
                    ).then_inc(red_sem, 1)
                # Epilogue.
                vector.wait_ge(act_sem, 3)
                vector.reduce_sum(
                    ss[:, :], es[:, :], axis=mybir.AxisListType.X
                ).then_inc(red_sem, 1)
                vector.wait_ge(red_sem, NST + 1)
                vector.reciprocal(rs[:, :], ss[:, :]).then_inc(red_sem, 1)
                vector.wait_ge(red_sem, NST + 2)
                vector.tensor_scalar_mul(a1s[:, :], es[:, :], rs[:, 0:1]).then_inc(
                    red_sem, 1
                )
                vector.wait_ge(red_sem, NST + 3)
                vector.tensor_mul(t1s[:, :], y1s[:, :], a1s[:, :]).then_inc(red_sem, 1)
                vector.wait_ge(pe_sem, 9)
                vector.wait_ge(red_sem, NST + 4)
                vector.tensor_add(y2s[:, :], t1s[:, :], p2_ps[:, :]).then_inc(
                    red_sem, 1
                )
                vector.wait_ge(red_sem, NST + 5)
                vector.tensor_scalar(
                    y3s[:, :],
                    y2s[:, :],
                    scals[:, 1:2],
                    0.0,
                    op0=mybir.AluOpType.mult,
                    op1=mybir.AluOpType.max,
                ).then_inc(red_sem, 1)

            @block.tensor
            def _(tensor):
                tensor.wait_ge(param_sem, 80)
                tensor.wait_ge(red_sem, NST)
                # y1T[h, b] += sum_c W1T[c, h] * yT[c, b]
                for ct in range(CT):
                    tensor.matmul(
                        y1t_ps[:, :],
                        w1ts[:, ct, :],
                        yt[:, ct, :],
                        start=(ct == 0),
                        stop=(ct == CT - 1),
                    ).then_inc(pe_sem, 1)
                # y1[b, h] += sum_c yT[c, b] * W1T[c, h]
                for ct in range(CT):
                    tensor.matmul(
                        y1_ps[:, :],
                        yt[:, ct, :],
                        w1ts[:, ct, :],
                        start=(ct == 0),
                        stop=(ct == CT - 1),
                    ).then_inc(pe_sem, 1)
                # p2[b, k] = sum_h y1T[h, b] * A2[h, k]
                tensor.wait_ge(act_sem, 2)
                tensor.matmul(
                    p2_ps[:, :], y1ts[:, :], a2s[:, :], start=True, stop=True
                ).then_inc(pe_sem, 1)
                # y3T = transpose(y3)
                tensor.wait_ge(red_sem, NST + 6)
                tensor.transpose(y3t_ps[:, :], y3s[:, :], eyes[:, :]).then_inc(
                    pe_sem, 1
                )
                # out[b, o] = sum_h y3T[h, b] * W4T[h, o]
                tensor.wait_ge(act_sem, 4)
                tensor.matmul(
                    o_ps[:, :], y3ts[:, :], w4ts[:, :], start=True, stop=True
                ).then_inc(pe_sem, 1)

    return nc


def prep_in_maps(x, W1, A2, w2, w3, W4, hw: int = HW):
    """Shard x over batch; replicate (pre-transposed) params."""
    x = np.ascontiguousarray(np.asarray(x, dtype=np.float32))
    # W1T with the mean scale folded in: [c, h] -> [128, CT, HIDE] with
    # w1t[p, ct, h] = W1[h, ct*128+p] / hw
    w1t = np.ascontiguousarray(
        (np.asarray(W1, np.float32).T / hw).reshape(CT, 128, HIDE).transpose(1, 0, 2)
    )
    a2 = np.ascontiguousarray(np.asarray(A2, np.float32))
    w4t = np.ascontiguousarray(np.asarray(W4, np.float32).T)
    scal = np.empty((BL, 2), np.float32)
    scal[:, 0] = np.float32(w2)
    scal[:, 1] = np.float32(w3)
    eye8 = np.eye(BL, dtype=np.float32)

    in_maps = []
    for c in range(NCORES):
        xs = x[c * BL:(c + 1) * BL].reshape(BL, CT, 128, hw)
        in_maps.append(
            {
                "x": xs,
                "w1t": w1t,
                "a2": a2,
                "w4t": w4t,
                "scal": scal,
                "eye8": eye8,
            }
        )
    return in_maps


def run(inputs: dict, trace: bool = False, tmpdir: str | None = None):
    """Build + run on 8 cores. Returns (full_output, BassKernelResults)."""
    nc = build_nc()
    in_maps = prep_in_maps(
        inputs["x"], inputs["W1"], inputs["A2"], inputs["w2"], inputs["w3"],
        inputs["W4"],
    )
    res = run_bass_kernel_spmd(
        nc, in_maps, core_ids=list(range(NCORES)), trace=trace, tmpdir=tmpdir
    )
    out = np.concatenate([res.results[c]["out"] for c in range(NCORES)], axis=0)
    return out.reshape(B, OP, 1, 1).astype(np.float32), res


def kernel(**inputs) -> np.ndarray:
    out, _ = run(inputs, trace=False)
    return out


# revision 13
# speedup vs baseline: 11.7675x; 1.0069x over previous
"""AGCA (adaptive graph channel attention) distributed Bass kernel for TRN2.

Reference computation (per batch row b):
    y   = mean(x[b], axis=(H,W))                    # [CIN]
    y1  = W1 @ y                                    # [HIDE]
    A1  = softmax(w2 * y1)                          # [HIDE]
    y2  = y1 * A1 + A2.T-contract(y1)               # y1@A2
    y3  = relu(w3 * y2)
    out = sigmoid(W4 @ y3)                          # [OP]

Sharding: pure data-parallel over batch. Each of the 8 cores handles
B/8 = 8 batch rows end-to-end; the tiny params are replicated. No
collectives. The kernel is memory-bound on streaming x (64 MiB/core).

Per-core dataflow:
  - x shard viewed as [BL=8, CT=4, 128, 4096] (batch, channel-tile,
    channel-within-tile, H*W).
  - 16 "supertiles" of [128, 2, 4096] f32 (4 MiB) are DMAd (HWDGE via
    the sync engine, triple-buffered) and sum-reduced along the free
    axis by the vector engine into yT tiles [128c, 8b].
  - The 1/4096 mean scale is folded into W1 on the host.
  - Epilogue: small matmuls on the tensor engine (both y1 [8,128] and
    y1T [128,8] layouts are computed so softmax runs along the free
    axis), exp/sigmoid on the scalar engine, elementwise on vector.
  - Output [8, 512] (batch-major) DMAd out; host concatenates shards.
"""

import numpy as np

import concourse.bass as bass
import concourse.mybir as mybir
from concourse.bass_utils import run_bass_kernel_spmd

F32 = mybir.dt.float32

B, CIN, H, W = 64, 512, 64, 64
HW = H * W          # 4096
NCORES = 8
BL = B // NCORES    # 8 batch rows per core
CT = CIN // 128     # 4 channel tiles
HIDE = 128
OP = 512
NBST = 2            # batch rows per (full) supertile
NBUF = 3            # streaming buffers


def make_jobs(hw):
    """Streaming schedule: (b0, nb, ct, hw0, nhw, partial_idx|None).

    Body: 4 MiB supertiles. Tail tapers to 1-batch tiles and finally
    splits the very last tile's hw axis in half, so the last DVE reduce
    after the final DMA byte is ~2 us instead of ~9 us.
    """
    jobs = []
    for b0 in (0, 2, 4):
        for ct in range(CT):
            jobs.append((b0, 2, ct, 0, hw, None))
    for ct in range(CT):
        jobs.append((6, 1, ct, 0, hw, None))
    for ct in range(CT - 1):
        jobs.append((7, 1, ct, 0, hw, None))
    h2 = hw // 2
    jobs.append((7, 1, CT - 1, 0, h2, 0))
    jobs.append((7, 1, CT - 1, h2, hw - h2, 1))
    return jobs


def build_nc(hw: int = HW, nbuf: int = NBUF):
    nc = bass.Bass()

    x_e = nc.declare_dram_parameter("x", [BL, CT, 128, hw], F32, isOutput=False)
    w1t_e = nc.declare_dram_parameter("w1t", [128, CT, HIDE], F32, isOutput=False)
    a2_e = nc.declare_dram_parameter("a2", [HIDE, HIDE], F32, isOutput=False)
    w4t_e = nc.declare_dram_parameter("w4t", [HIDE, OP], F32, isOutput=False)
    scal_e = nc.declare_dram_parameter("scal", [BL, 2], F32, isOutput=False)
    eye_e = nc.declare_dram_parameter("eye8", [BL, BL], F32, isOutput=False)
    out_e = nc.declare_dram_parameter("out", [BL, OP], F32, isOutput=True)

    Exp = mybir.ActivationFunctionType.Exp
    Sigmoid = mybir.ActivationFunctionType.Sigmoid

    from contextlib import ExitStack

    with ExitStack() as ctx:
        bufs = [
            ctx.enter_context(nc.sbuf_tensor(f"buf{j}", [128, NBST, hw], F32))
            for j in range(nbuf)
        ]
        yt = ctx.enter_context(nc.sbuf_tensor("yt", [128, CT, BL], F32))
        w1ts = ctx.enter_context(nc.sbuf_tensor("w1ts", [128, CT, HIDE], F32))
        a2s = ctx.enter_context(nc.sbuf_tensor("a2s", [HIDE, HIDE], F32))
        w4ts = ctx.enter_context(nc.sbuf_tensor("w4ts", [HIDE, OP], F32))
        scals = ctx.enter_context(nc.sbuf_tensor("scals", [BL, 2], F32))
        eyes = ctx.enter_context(nc.sbuf_tensor("eyes", [BL, BL], F32))

        y1s = ctx.enter_context(nc.sbuf_tensor("y1s", [BL, HIDE], F32))
        y1ts = ctx.enter_context(nc.sbuf_tensor("y1ts", [HIDE, BL], F32))
        es = ctx.enter_context(nc.sbuf_tensor("es", [BL, HIDE], F32))
        ss = ctx.enter_context(nc.sbuf_tensor("ss", [BL, 1], F32))
        rs = ctx.enter_context(nc.sbuf_tensor("rs", [BL, 1], F32))
        a1s = ctx.enter_context(nc.sbuf_tensor("a1s", [BL, HIDE], F32))
        t1s = ctx.enter_context(nc.sbuf_tensor("t1s", [BL, HIDE], F32))
        y2s = ctx.enter_context(nc.sbuf_tensor("y2s", [BL, HIDE], F32))
        y3s = ctx.enter_context(nc.sbuf_tensor("y3s", [BL, HIDE], F32))
        y3ts = ctx.enter_context(nc.sbuf_tensor("y3ts", [HIDE, BL], F32))
        outs = ctx.enter_context(nc.sbuf_tensor("outs", [BL, OP], F32))

        ytx = ctx.enter_context(nc.sbuf_tensor("ytx", [128, 2], F32))
        de1 = ctx.enter_context(nc.sbuf_tensor("de1", [1, 1], F32))
        de2 = ctx.enter_context(nc.sbuf_tensor("de2", [1, 1], F32))

        y1_ps = ctx.enter_context(nc.psum_tensor("y1_ps", [BL, HIDE], F32))
        y1t_ps = ctx.enter_context(nc.psum_tensor("y1t_ps", [HIDE, BL], F32))
        p2_ps = ctx.enter_context(nc.psum_tensor("p2_ps", [BL, HIDE], F32))
        y3t_ps = ctx.enter_context(nc.psum_tensor("y3t_ps", [HIDE, BL], F32))
        o_ps = ctx.enter_context(nc.psum_tensor("o_ps", [BL, OP], F32))

        slot_sems = [
            ctx.enter_context(nc.semaphore(f"slot_sem{j}")) for j in range(nbuf)
        ]
        out_sem = ctx.enter_context(nc.semaphore("out_sem"))
        param_sem = ctx.enter_context(nc.semaphore("param_sem"))
        red_sem = ctx.enter_context(nc.semaphore("red_sem"))
        pe_sem = ctx.enter_context(nc.semaphore("pe_sem"))
        act_sem = ctx.enter_context(nc.semaphore("act_sem"))

        jobs = make_jobs(hw)
        njobs = len(jobs)
        R0 = njobs + 1  # red_sem count once yt is complete (incl. combine)

        with nc.Block() as block:

            @block.sync
            def _(sync):
                # Stream x; NBUF-deep buffer rotation.
                for i, (b0, nb, ct, hw0, nhw, _pidx) in enumerate(jobs):
                    if i >= nbuf:
                        sync.wait_ge(red_sem, i - nbuf + 1)
                    src = x_e[b0:b0 + nb, ct, :, hw0:hw0 + nhw].rearrange(
                        "b p w -> p b w"
                    )
                    sync.dma_start(
                        out=bufs[i % nbuf][:, 0:nb, 0:nhw], in_=src
                    ).then_inc(slot_sems[i % nbuf], 16)
                # Output DMA after the sigmoid lands in SBUF.
                sync.wait_ge(act_sem, 4)
                sync.dma_start(out=out_e[:, :], in_=outs[:, :]).then_inc(out_sem, 16)
                sync.wait_ge(out_sem, 16)

            @block.scalar
            def _(scalar):
                # Param loads ride the ACT HWDGE queue so they don't delay
                # the x stream on the sync queue.
                scalar.dma_start(out=w1ts[:, :, :], in_=w1t_e[:, :, :]).then_inc(
                    param_sem, 16
                )
                scalar.dma_start(out=a2s[:, :], in_=a2_e[:, :]).then_inc(param_sem, 16)
                scalar.dma_start(out=w4ts[:, :], in_=w4t_e[:, :]).then_inc(
                    param_sem, 16
                )
                scalar.dma_start(out=scals[:, :], in_=scal_e[:, :]).then_inc(
                    param_sem, 16
                )
                scalar.dma_start(out=eyes[:, :], in_=eye_e[:, :]).then_inc(
                    param_sem, 16
                )
                # Preload ACT LUTs (exp->sel0, sigmoid->sel1) during the
                # stream so the epilogue's real calls hit resident tables.
                c0 = nc.const_aps.tensor(0.0, (1, 1))
                scalar.activation(de1[:, :], c0, Exp)
                scalar.activation(de2[:, :], c0, Sigmoid)
                # Epilogue.
                scalar.wait_ge(pe_sem, 8)
                scalar.copy(y1s[:, :], y1_ps[:, :]).then_inc(act_sem, 1)
                scalar.wait_ge(param_sem, 80)
                scalar.wait_ge(act_sem, 1)  # y1s write retired (same-engine)
                scalar.activation(
                    es[:, :], y1s[:, :], Exp, scale=scals[:, 0:1]
                ).then_inc(act_sem, 1)
                scalar.wait_ge(pe_sem, 10)
                scalar.copy(y3ts[:, :], y3t_ps[:, :]).then_inc(act_sem, 1)
                scalar.wait_ge(pe_sem, 11)
                scalar.activation(outs[:, :], o_ps[:, :], Sigmoid).then_inc(act_sem, 1)

            @block.vector
            def _(vector):
                for i, (b0, nb, ct, hw0, nhw, pidx) in enumerate(jobs):
                    vector.wait_ge(slot_sems[i % nbuf], 16 * (i // nbuf + 1))
                    dst = (
                        yt[:, ct, b0:b0 + nb]
                        if pidx is None
                        else ytx[:, pidx:pidx + 1]
                    )
                    vector.reduce_sum(
                        dst,
                        bufs[i % nbuf][:, 0:nb, 0:nhw],
                        axis=mybir.AxisListType.X,
                    ).then_inc(red_sem, 1)
                # Combine the split last tile: yt[:, CT-1, BL-1] = ytx0 + ytx1
                vector.wait_ge(red_sem, njobs)
                vector.tensor_add(
                    yt[:, CT - 1, BL - 1:BL], ytx[:, 0:1], ytx[:, 1:2]
                ).then_inc(red_sem, 1)
                # Epilogue. y1ts copy runs on DVE, parallel to ACT's y1s copy.
                vector.wait_ge(pe_sem, 4)
                vector.tensor_copy(y1ts[:, :], y1t_ps[:, :]).then_inc(red_sem, 1)
                vector.wait_ge(act_sem, 2)
                vector.reduce_sum(
                    ss[:, :], es[:, :], axis=mybir.AxisListType.X
                ).then_inc(red_sem, 1)
                vector.wait_ge(red_sem, R0 + 2)
                vector.reciprocal(rs[:, :], ss[:, :]).then_inc(red_sem, 1)
                vector.wait_ge(red_sem, R0 + 3)
                vector.tensor_scalar_mul(a1s[:, :], es[:, :], rs[:, 0:1]).then_inc(
                    red_sem, 1
                )
                vector.wait_ge(red_sem, R0 + 4)
                vector.tensor_mul(t1s[:, :], y1s[:, :], a1s[:, :]).then_inc(red_sem, 1)
                vector.wait_ge(pe_sem, 9)
                vector.wait_ge(red_sem, R0 + 5)
                vector.tensor_add(y2s[:, :], t1s[:, :], p2_ps[:, :]).then_inc(
                    red_sem, 1
                )
                vector.wait_ge(red_sem, R0 + 6)
                vector.tensor_scalar(
                    y3s[:, :],
                    y2s[:, :],
                    scals[:, 1:2],
                    0.0,
                    op0=mybir.AluOpType.mult,
                    op1=mybir.AluOpType.max,
                ).then_inc(red_sem, 1)

            @block.tensor
            def _(tensor):
                tensor.wait_ge(param_sem, 80)
                tensor.wait_ge(red_sem, R0)
                # y1T[h, b] += sum_c W1T[c, h] * yT[c, b]
                for ct in range(CT):
                    tensor.matmul(
                        y1t_ps[:, :],
                        w1ts[:, ct, :],
                        yt[:, ct, :],
                        start=(ct == 0),
                        stop=(ct == CT - 1),
                    ).then_inc(pe_sem, 1)
                # y1[b, h] += sum_c yT[c, b] * W1T[c, h]
                for ct in range(CT):
                    tensor.matmul(
                        y1_ps[:, :],
                        yt[:, ct, :],
                        w1ts[:, ct, :],
                        start=(ct == 0),
                        stop=(ct == CT - 1),
                    ).then_inc(pe_sem, 1)
                # p2[b, k] = sum_h y1T[h, b] * A2[h, k]
                tensor.wait_ge(red_sem, R0 + 1)
                tensor.matmul(
                    p2_ps[:, :], y1ts[:, :], a2s[:, :], start=True, stop=True
                ).then_inc(pe_sem, 1)
                # y3T = transpose(y3)
                tensor.wait_ge(red_sem, R0 + 7)
                tensor.transpose(y3t_ps[:, :], y3s[:, :], eyes[:, :]).then_inc(
                    pe_sem, 1
                )
                # out[b, o] = sum_h y3T[h, b] * W4T[h, o]
                tensor.wait_ge(act_sem, 3)
                tensor.matmul(
                    o_ps[:, :], y3ts[:, :], w4ts[:, :], start=True, stop=True
                ).then_inc(pe_sem, 1)

    return nc


def prep_in_maps(x, W1, A2, w2, w3, W4, hw: int = HW):
    """Shard x over batch; replicate (pre-transposed) params."""
    x = np.ascontiguousarray(np.asarray(x, dtype=np.float32))
    # W1T with the mean scale folded in: [c, h] -> [128, CT, HIDE] with
    # w1t[p, ct, h] = W1[h, ct*128+p] / hw
    w1t = np.ascontiguousarray(
        (np.asarray(W1, np.float32).T / hw).reshape(CT, 128, HIDE).transpose(1, 0, 2)
    )
    a2 = np.ascontiguousarray(np.asarray(A2, np.float32))
    w4t = np.ascontiguousarray(np.asarray(W4, np.float32).T)
    scal = np.empty((BL, 2), np.float32)
    scal[:, 0] = np.float32(w2)
    scal[:, 1] = np.float32(w3)
    eye8 = np.eye(BL, dtype=np.float32)

    in_maps = []
    for c in range(NCORES):
        xs = x[c * BL:(c + 1) * BL].reshape(BL, CT, 128, hw)
        in_maps.append(
            {
                "x": xs,
                "w1t": w1t,
                "a2": a2,
                "w4t": w4t,
                "scal": scal,
                "eye8": eye8,
            }
        )
    return in_maps


def run(inputs: dict, trace: bool = False, tmpdir: str | None = None):
    """Build + run on 8 cores. Returns (full_output, BassKernelResults)."""
    nc = build_nc()
    in_maps = prep_in_maps(
        inputs["x"], inputs["W1"], inputs["A2"], inputs["w2"], inputs["w3"],
        inputs["W4"],
    )
    res = run_bass_kernel_spmd(
        nc, in_maps, core_ids=list(range(NCORES)), trace=trace, tmpdir=tmpdir
    )
    out = np.concatenate([res.results[c]["out"] for c in range(NCORES)], axis=0)
    return out.reshape(B, OP, 1, 1).astype(np.float32), res


def kernel(**inputs) -> np.ndarray:
    out, _ = run(inputs, trace=False)
    return out


# revision 17
# speedup vs baseline: 14.1162x; 1.1996x over previous
"""AGCA (adaptive graph channel attention) distributed Bass kernel for TRN2.

Reference computation (per batch row b):
    y   = mean(x[b], axis=(H,W))                    # [CIN]
    y1  = W1 @ y                                    # [HIDE]
    A1  = softmax(w2 * y1)                          # [HIDE]
    y2  = y1 * A1 + A2.T-contract(y1)               # y1@A2
    y3  = relu(w3 * y2)
    out = sigmoid(W4 @ y3)                          # [OP]

Sharding: pure data-parallel over batch. Each of the 8 cores handles
B/8 = 8 batch rows end-to-end; the tiny params are replicated. No
collectives. The kernel is memory-bound on streaming x (64 MiB/core).

Per-core dataflow:
  - x shard viewed as [BL=8, CT=4, 128, 4096] (batch, channel-tile,
    channel-within-tile, H*W).
  - 16 "supertiles" of [128, 2, 4096] f32 (4 MiB) are DMAd (HWDGE via
    the sync engine, triple-buffered) and sum-reduced along the free
    axis by the vector engine into yT tiles [128c, 8b].
  - The 1/4096 mean scale is folded into W1 on the host.
  - Epilogue: small matmuls on the tensor engine (both y1 [8,128] and
    y1T [128,8] layouts are computed so softmax runs along the free
    axis), exp/sigmoid on the scalar engine, elementwise on vector.
  - Output [8, 512] (batch-major) DMAd out; host concatenates shards.
"""

import numpy as np

import concourse.bass as bass
import concourse.mybir as mybir
from concourse.bass_utils import run_bass_kernel_spmd

F32 = mybir.dt.float32

B, CIN, H, W = 64, 512, 64, 64
HW = H * W          # 4096
NCORES = 8
BL = B // NCORES    # 8 batch rows per core
CT = CIN // 128     # 4 channel tiles
HIDE = 128
OP = 512
NBST = 2            # batch rows per (full) supertile
NBUF = 3            # streaming buffers


NTAPER = 5  # geometric hw-split chunks for the very last batch row


def make_jobs(hw):
    """Streaming schedule: (b0, nb, ct, hw0, nhw, partial_idx|None).

    ct-major order so each channel tile's W1 matmuls can run mid-stream.
    Body: 4 MiB supertiles. The very last tile (ct=CT-1, b=BL-1) tapers
    geometrically along hw so the post-last-byte reduce work is ~0.5 us.
    Returns (jobs, ct_done) where ct_done[ct] = red_sem count at which
    that channel tile of yt is fully reduced (ct=CT-1 needs the combine).
    """
    jobs = []
    ct_done = []
    for ct in range(CT):
        if ct < CT - 1:
            for b0 in range(0, BL, NBST):
                jobs.append((b0, NBST, ct, 0, hw, None))
        else:
            for b0 in range(0, BL - NBST, NBST):
                jobs.append((b0, NBST, ct, 0, hw, None))
            jobs.append((BL - 2, 1, ct, 0, hw, None))
            chunks = [hw // 2, hw // 4, hw // 8, hw // 16, hw // 16]
            assert len(chunks) == NTAPER and sum(chunks) == hw
            hw0 = 0
            for k, c in enumerate(chunks):
                jobs.append((BL - 1, 1, ct, hw0, c, k))
                hw0 += c
        ct_done.append(len(jobs))
    return jobs, ct_done


def build_nc(hw: int = HW, nbuf: int = NBUF):
    nc = bass.Bass()
    BF16 = mybir.dt.bfloat16

    x_e = nc.declare_dram_parameter("x", [BL, CT, 128, hw], F32, isOutput=False)
    w1t_e = nc.declare_dram_parameter("w1t", [128, CT, HIDE], F32, isOutput=False)
    a2_e = nc.declare_dram_parameter("a2", [HIDE, HIDE], BF16, isOutput=False)
    w4t_e = nc.declare_dram_parameter("w4t", [HIDE, OP], BF16, isOutput=False)
    scal_e = nc.declare_dram_parameter("scal", [BL, 2], F32, isOutput=False)
    eye_e = nc.declare_dram_parameter("eye8", [BL, BL], F32, isOutput=False)
    out_e = nc.declare_dram_parameter("out", [BL, OP], F32, isOutput=True)

    Exp = mybir.ActivationFunctionType.Exp

    from contextlib import ExitStack

    with ExitStack() as ctx:
        bufs = [
            ctx.enter_context(nc.sbuf_tensor(f"buf{j}", [128, NBST, hw], F32))
            for j in range(nbuf)
        ]
        yt = ctx.enter_context(nc.sbuf_tensor("yt", [128, CT, BL], F32))
        ytx = ctx.enter_context(nc.sbuf_tensor("ytx", [128, NTAPER], F32))
        w1ts = ctx.enter_context(nc.sbuf_tensor("w1ts", [128, CT, HIDE], F32))
        a2s = ctx.enter_context(nc.sbuf_tensor("a2s", [HIDE, HIDE], BF16))
        w4ts = ctx.enter_context(nc.sbuf_tensor("w4ts", [HIDE, OP], BF16))
        scals = ctx.enter_context(nc.sbuf_tensor("scals", [BL, 2], F32))
        eyes = ctx.enter_context(nc.sbuf_tensor("eyes", [BL, BL], F32))
        de1 = ctx.enter_context(nc.sbuf_tensor("de1", [1, 1], F32))

        y1ts = ctx.enter_context(nc.sbuf_tensor("y1ts", [HIDE, BL], BF16))
        es = ctx.enter_context(nc.sbuf_tensor("es", [BL, HIDE], F32))
        ss = ctx.enter_context(nc.sbuf_tensor("ss", [BL, 1], F32))
        rs = ctx.enter_context(nc.sbuf_tensor("rs", [BL, 1], F32))
        t1s = ctx.enter_context(nc.sbuf_tensor("t1s", [BL, HIDE], F32))
        y2s = ctx.enter_context(nc.sbuf_tensor("y2s", [BL, HIDE], F32))
        y3s = ctx.enter_context(nc.sbuf_tensor("y3s", [BL, HIDE], F32))
        y3ts = ctx.enter_context(nc.sbuf_tensor("y3ts", [HIDE, BL], BF16))
        esig = ctx.enter_context(nc.sbuf_tensor("esig", [BL, OP], F32))
        op1s = ctx.enter_context(nc.sbuf_tensor("op1s", [BL, OP], F32))
        outs = ctx.enter_context(nc.sbuf_tensor("outs", [BL, OP], F32))

        y1_ps = ctx.enter_context(nc.psum_tensor("y1_ps", [BL, HIDE], F32))
        y1t_ps = ctx.enter_context(nc.psum_tensor("y1t_ps", [HIDE, BL], F32))
        p2_ps = ctx.enter_context(nc.psum_tensor("p2_ps", [BL, HIDE], F32))
        y3t_ps = ctx.enter_context(nc.psum_tensor("y3t_ps", [HIDE, BL], F32))
        o_ps = ctx.enter_context(nc.psum_tensor("o_ps", [BL, OP], F32))

        slot_sems = [
            ctx.enter_context(nc.semaphore(f"slot_sem{j}")) for j in range(nbuf)
        ]
        out_sem = ctx.enter_context(nc.semaphore("out_sem"))
        param_sem = ctx.enter_context(nc.semaphore("param_sem"))
        red_sem = ctx.enter_context(nc.semaphore("red_sem"))
        pe_sem = ctx.enter_context(nc.semaphore("pe_sem"))
        act_sem = ctx.enter_context(nc.semaphore("act_sem"))

        jobs, ct_done = make_jobs(hw)
        njobs = len(jobs)
        R0 = njobs + 1  # red_sem count once yt is complete (incl. combine)

        with nc.Block() as block:

            @block.sync
            def _(sync):
                # Stream x; NBUF-deep buffer rotation.
                for i, (b0, nb, ct, hw0, nhw, _pidx) in enumerate(jobs):
                    if i >= nbuf:
                        sync.wait_ge(red_sem, i - nbuf + 1)
                    src = x_e[b0:b0 + nb, ct, :, hw0:hw0 + nhw].rearrange(
                        "b p w -> p b w"
                    )
                    sync.dma_start(
                        out=bufs[i % nbuf][:, 0:nb, 0:nhw], in_=src
                    ).then_inc(slot_sems[i % nbuf], 16)
                # Output DMA once DVE finishes the sigmoid reciprocal.
                sync.wait_ge(red_sem, R0 + 7)
                sync.dma_start(out=out_e[:, :], in_=outs[:, :]).then_inc(out_sem, 16)
                sync.wait_ge(out_sem, 16)

            @block.scalar
            def _(scalar):
                # Param loads ride the ACT HWDGE queue so they don't delay
                # the x stream on the sync queue.
                scalar.dma_start(out=w1ts[:, :, :], in_=w1t_e[:, :, :]).then_inc(
                    param_sem, 16
                )
                scalar.dma_start(out=a2s[:, :], in_=a2_e[:, :]).then_inc(param_sem, 16)
                scalar.dma_start(out=w4ts[:, :], in_=w4t_e[:, :]).then_inc(
                    param_sem, 16
                )
                scalar.dma_start(out=scals[:, :], in_=scal_e[:, :]).then_inc(
                    param_sem, 16
                )
                scalar.dma_start(out=eyes[:, :], in_=eye_e[:, :]).then_inc(
                    param_sem, 16
                )
                # Preload the exp table set during the stream; every ACT
                # transcendental below is exp, so no tail table loads.
                c0 = nc.const_aps.tensor(0.0, (1, 1))
                scalar.activation(de1[:, :], c0, Exp)
                # Epilogue: exp(w2*y1) with fused free-dim sum (softmax denom),
                # reading y1 straight out of PSUM.
                scalar.wait_ge(param_sem, 80)
                scalar.wait_ge(pe_sem, 8)
                scalar.activation(
                    es[:, :], y1_ps[:, :], Exp, scale=scals[:, 0:1],
                    accum_out=ss[:, :],
                ).then_inc(act_sem, 1)
                scalar.wait_ge(pe_sem, 10)
                scalar.copy(y3ts[:, :], y3t_ps[:, :]).then_inc(act_sem, 1)
                # sigmoid(z) = 1/(1+exp(-z)): exp(-z) on ACT (table resident),
                # the +1 and reciprocal on DVE.
                scalar.wait_ge(pe_sem, 11)
                scalar.activation(
                    esig[:, :], o_ps[:, :], Exp, scale=-1.0
                ).then_inc(act_sem, 1)

            @block.vector
            def _(vector):
                for i, (b0, nb, ct, hw0, nhw, pidx) in enumerate(jobs):
                    vector.wait_ge(slot_sems[i % nbuf], 16 * (i // nbuf + 1))
                    dst = (
                        yt[:, ct, b0:b0 + nb]
                        if pidx is None
                        else ytx[:, pidx:pidx + 1]
                    )
                    vector.reduce_sum(
                        dst,
                        bufs[i % nbuf][:, 0:nb, 0:nhw],
                        axis=mybir.AxisListType.X,
                    ).then_inc(red_sem, 1)
                # Combine the tapered chunks: yt[:, CT-1, BL-1] = sum(ytx)
                vector.wait_ge(red_sem, njobs)
                vector.reduce_sum(
                    yt[:, CT - 1, BL - 1:BL], ytx[:, :],
                    axis=mybir.AxisListType.X,
                ).then_inc(red_sem, 1)
                # Epilogue.
                vector.wait_ge(pe_sem, 7)
                vector.tensor_copy(y1ts[:, :], y1t_ps[:, :]).then_inc(red_sem, 1)
                vector.wait_ge(act_sem, 1)
                vector.reciprocal(rs[:, :], ss[:, :]).then_inc(red_sem, 1)
                vector.wait_ge(red_sem, R0 + 2)
                # t1 = (es * 1/s) * y1  (y1 read from PSUM)
                vector.scalar_tensor_tensor(
                    t1s[:, :], es[:, :], rs[:, 0:1], y1_ps[:, :],
                    op0=mybir.AluOpType.mult, op1=mybir.AluOpType.mult,
                ).then_inc(red_sem, 1)
                vector.wait_ge(pe_sem, 9)
                vector.wait_ge(red_sem, R0 + 3)
                vector.tensor_add(y2s[:, :], t1s[:, :], p2_ps[:, :]).then_inc(
                    red_sem, 1
                )
                vector.wait_ge(red_sem, R0 + 4)
                vector.tensor_scalar(
                    y3s[:, :],
                    y2s[:, :],
                    scals[:, 1:2],
                    0.0,
                    op0=mybir.AluOpType.mult,
                    op1=mybir.AluOpType.max,
                ).then_inc(red_sem, 1)
                # Final sigmoid tail: outs = 1/(1+esig)
                vector.wait_ge(act_sem, 3)
                vector.tensor_scalar_add(op1s[:, :], esig[:, :], 1.0).then_inc(
                    red_sem, 1
                )
                vector.wait_ge(red_sem, R0 + 6)
                vector.reciprocal(outs[:, :], op1s[:, :]).then_inc(red_sem, 1)

            @block.tensor
            def _(tensor):
                tensor.wait_ge(param_sem, 80)
                # W1 matmuls per channel tile, issued as soon as that tile of
                # yt is fully reduced (overlaps the remaining stream).
                for ct in range(CT):
                    done = R0 if ct == CT - 1 else ct_done[ct]
                    tensor.wait_ge(red_sem, done)
                    tensor.matmul(
                        y1t_ps[:, :],
                        w1ts[:, ct, :],
                        yt[:, ct, :],
                        start=(ct == 0),
                        stop=(ct == CT - 1),
                    ).then_inc(pe_sem, 1)
                    tensor.matmul(
                        y1_ps[:, :],
                        yt[:, ct, :],
                        w1ts[:, ct, :],
                        start=(ct == 0),
                        stop=(ct == CT - 1),
                    ).then_inc(pe_sem, 1)
                # p2[b, k] = sum_h y1T[h, b] * A2[h, k]
                tensor.wait_ge(red_sem, R0 + 1)
                tensor.matmul(
                    p2_ps[:, :], y1ts[:, :], a2s[:, :], start=True, stop=True
                ).then_inc(pe_sem, 1)
                # y3T = transpose(y3)
                tensor.wait_ge(red_sem, R0 + 5)
                tensor.transpose(y3t_ps[:, :], y3s[:, :], eyes[:, :]).then_inc(
                    pe_sem, 1
                )
                # out[b, o] = sum_h y3T[h, b] * W4T[h, o]
                tensor.wait_ge(act_sem, 2)
                tensor.matmul(
                    o_ps[:, :], y3ts[:, :], w4ts[:, :], start=True, stop=True
                ).then_inc(pe_sem, 1)

    return nc


def prep_in_maps(x, W1, A2, w2, w3, W4, hw: int = HW):
    """Shard x over batch; replicate (pre-transposed) params."""
    x = np.ascontiguousarray(np.asarray(x, dtype=np.float32))
    # W1T with the mean scale folded in: [c, h] -> [128, CT, HIDE] with
    # w1t[p, ct, h] = W1[h, ct*128+p] / hw
    w1t = np.ascontiguousarray(
        (np.asarray(W1, np.float32).T / hw).reshape(CT, 128, HIDE).transpose(1, 0, 2)
    )
    import ml_dtypes

    a2 = np.ascontiguousarray(np.asarray(A2, np.float32)).astype(ml_dtypes.bfloat16)
    w4t = np.ascontiguousarray(np.asarray(W4, np.float32).T).astype(
        ml_dtypes.bfloat16
    )
    scal = np.empty((BL, 2), np.float32)
    scal[:, 0] = np.float32(w2)
    scal[:, 1] = np.float32(w3)
    eye8 = np.eye(BL, dtype=np.float32)

    in_maps = []
    for c in range(NCORES):
        xs = x[c * BL:(c + 1) * BL].reshape(BL, CT, 128, hw)
        in_maps.append(
            {
                "x": xs,
                "w1t": w1t,
                "a2": a2,
                "w4t": w4t,
                "scal": scal,
                "eye8": eye8,
            }
        )
    return in_maps


def run(inputs: dict, trace: bool = False, tmpdir: str | None = None):
    """Build + run on 8 cores. Returns (full_output, BassKernelResults)."""
    nc = build_nc()
    in_maps = prep_in_maps(
        inputs["x"], inputs["W1"], inputs["A2"], inputs["w2"], inputs["w3"],
        inputs["W4"],
    )
    res = run_bass_kernel_spmd(
        nc, in_maps, core_ids=list(range(NCORES)), trace=trace, tmpdir=tmpdir
    )
    out = np.concatenate([res.results[c]["out"] for c in range(NCORES)], axis=0)
    return out.reshape(B, OP, 1, 1).astype(np.float32), res


def kernel(**inputs) -> np.ndarray:
    out, _ = run(inputs, trace=False)
    return out


# revision 20
# speedup vs baseline: 14.4886x; 1.0264x over previous
"""AGCA (adaptive graph channel attention) distributed Bass kernel for TRN2.

Reference computation (per batch row b):
    y   = mean(x[b], axis=(H,W))                    # [CIN]
    y1  = W1 @ y                                    # [HIDE]
    A1  = softmax(w2 * y1)                          # [HIDE]
    y2  = y1 * A1 + A2.T-contract(y1)               # y1@A2
    y3  = relu(w3 * y2)
    out = sigmoid(W4 @ y3)                          # [OP]

Sharding: pure data-parallel over batch. Each of the 8 cores handles
B/8 = 8 batch rows end-to-end; the tiny params are replicated. No
collectives. The kernel is memory-bound on streaming x (64 MiB/core).

Per-core dataflow:
  - x shard viewed as [BL=8, CT=4, 128, 4096] (batch, channel-tile,
    channel-within-tile, H*W).
  - 16 "supertiles" of [128, 2, 4096] f32 (4 MiB) are DMAd (HWDGE via
    the sync engine, triple-buffered) and sum-reduced along the free
    axis by the vector engine into yT tiles [128c, 8b].
  - The 1/4096 mean scale is folded into W1 on the host.
  - Epilogue: small matmuls on the tensor engine (both y1 [8,128] and
    y1T [128,8] layouts are computed so softmax runs along the free
    axis), exp/sigmoid on the scalar engine, elementwise on vector.
  - Output [8, 512] (batch-major) DMAd out; host concatenates shards.
"""

import numpy as np

import concourse.bass as bass
import concourse.mybir as mybir
from concourse.bass_utils import run_bass_kernel_spmd

F32 = mybir.dt.float32

B, CIN, H, W = 64, 512, 64, 64
HW = H * W          # 4096
NCORES = 8
BL = B // NCORES    # 8 batch rows per core
CT = CIN // 128     # 4 channel tiles
HIDE = 128
OP = 512
NBST = 2            # batch rows per (full) supertile
NBUF = 5            # streaming buffers


NTAPER = 5  # geometric hw-split chunks for the very last batch row


def make_jobs(hw):
    """Streaming schedule: (b0, nb, ct, hw0, nhw, partial_idx|None).

    ct-major order so each channel tile's W1 matmuls can run mid-stream.
    Body: 4 MiB supertiles. The very last tile (ct=CT-1, b=BL-1) tapers
    geometrically along hw so the post-last-byte reduce work is ~0.5 us.
    Returns (jobs, ct_done) where ct_done[ct] = red_sem count at which
    that channel tile of yt is fully reduced (ct=CT-1 needs the combine).
    """
    jobs = []
    ct_done = []
    for ct in range(CT):
        if ct < CT - 1:
            for b0 in range(0, BL, NBST):
                jobs.append((b0, NBST, ct, 0, hw, None))
        else:
            for b0 in range(0, BL - NBST, NBST):
                jobs.append((b0, NBST, ct, 0, hw, None))
            jobs.append((BL - 2, 1, ct, 0, hw, None))
            chunks = [hw // 2, hw // 4, hw // 8, hw // 16, hw // 16]
            assert len(chunks) == NTAPER and sum(chunks) == hw
            hw0 = 0
            for k, c in enumerate(chunks):
                jobs.append((BL - 1, 1, ct, hw0, c, k))
                hw0 += c
        ct_done.append(len(jobs))
    return jobs, ct_done


def build_nc(hw: int = HW, nbuf: int = NBUF):
    nc = bass.Bass()
    BF16 = mybir.dt.bfloat16

    x_e = nc.declare_dram_parameter("x", [BL, CT, 128, hw], F32, isOutput=False)
    w1t_e = nc.declare_dram_parameter("w1t", [128, CT, HIDE], F32, isOutput=False)
    a2_e = nc.declare_dram_parameter("a2", [HIDE, HIDE], BF16, isOutput=False)
    w4t_e = nc.declare_dram_parameter("w4t", [HIDE, OP], BF16, isOutput=False)
    scal_e = nc.declare_dram_parameter("scal", [BL, 2], F32, isOutput=False)
    eye_e = nc.declare_dram_parameter("eye8", [BL, BL], F32, isOutput=False)
    out_e = nc.declare_dram_parameter("out", [BL, OP], F32, isOutput=True)

    Exp = mybir.ActivationFunctionType.Exp

    from contextlib import ExitStack

    with ExitStack() as ctx:
        bufs = [
            ctx.enter_context(nc.sbuf_tensor(f"buf{j}", [128, NBST, hw], F32))
            for j in range(nbuf)
        ]
        yt = ctx.enter_context(nc.sbuf_tensor("yt", [128, CT, BL], F32))
        ytx = ctx.enter_context(nc.sbuf_tensor("ytx", [128, NTAPER], F32))
        w1ts = ctx.enter_context(nc.sbuf_tensor("w1ts", [128, CT, HIDE], F32))
        a2s = ctx.enter_context(nc.sbuf_tensor("a2s", [HIDE, HIDE], BF16))
        w4ts = ctx.enter_context(nc.sbuf_tensor("w4ts", [HIDE, OP], BF16))
        scals = ctx.enter_context(nc.sbuf_tensor("scals", [BL, 2], F32))
        eyes = ctx.enter_context(nc.sbuf_tensor("eyes", [BL, BL], F32))
        de1 = ctx.enter_context(nc.sbuf_tensor("de1", [1, 1], F32))

        y1ts = ctx.enter_context(nc.sbuf_tensor("y1ts", [HIDE, BL], BF16))
        es = ctx.enter_context(nc.sbuf_tensor("es", [BL, HIDE], F32))
        ss = ctx.enter_context(nc.sbuf_tensor("ss", [BL, 1], F32))
        rs = ctx.enter_context(nc.sbuf_tensor("rs", [BL, 1], F32))
        t1s = ctx.enter_context(nc.sbuf_tensor("t1s", [BL, HIDE], F32))
        y2s = ctx.enter_context(nc.sbuf_tensor("y2s", [BL, HIDE], F32))
        y3s = ctx.enter_context(nc.sbuf_tensor("y3s", [BL, HIDE], F32))
        y3ts = ctx.enter_context(nc.sbuf_tensor("y3ts", [HIDE, BL], BF16))
        esig = ctx.enter_context(nc.sbuf_tensor("esig", [BL, OP], F32))
        op1s = ctx.enter_context(nc.sbuf_tensor("op1s", [BL, OP], F32))
        outs = ctx.enter_context(nc.sbuf_tensor("outs", [BL, OP], F32))

        y1_ps = ctx.enter_context(nc.psum_tensor("y1_ps", [BL, HIDE], F32))
        y1t_ps = ctx.enter_context(nc.psum_tensor("y1t_ps", [HIDE, BL], F32))
        p2_ps = ctx.enter_context(nc.psum_tensor("p2_ps", [BL, HIDE], F32))
        y3t_ps = ctx.enter_context(nc.psum_tensor("y3t_ps", [HIDE, BL], F32))
        o_ps = ctx.enter_context(nc.psum_tensor("o_ps", [BL, OP], F32))

        slot_sems = [
            ctx.enter_context(nc.semaphore(f"slot_sem{j}")) for j in range(nbuf)
        ]
        out_sem = ctx.enter_context(nc.semaphore("out_sem"))
        param_sem = ctx.enter_context(nc.semaphore("param_sem"))
        red_sem = ctx.enter_context(nc.semaphore("red_sem"))
        pe_sem = ctx.enter_context(nc.semaphore("pe_sem"))
        act_sem = ctx.enter_context(nc.semaphore("act_sem"))

        jobs, ct_done = make_jobs(hw)
        njobs = len(jobs)
        R0 = njobs + 1  # red_sem count once yt is complete (incl. combine)

        with nc.Block() as block:

            @block.sync
            def _(sync):
                # Stream x; NBUF-deep buffer rotation.
                for i, (b0, nb, ct, hw0, nhw, _pidx) in enumerate(jobs):
                    if i >= nbuf:
                        sync.wait_ge(red_sem, i - nbuf + 1)
                    src = x_e[b0:b0 + nb, ct, :, hw0:hw0 + nhw].rearrange(
                        "b p w -> p b w"
                    )
                    sync.dma_start(
                        out=bufs[i % nbuf][:, 0:nb, 0:nhw], in_=src
                    ).then_inc(slot_sems[i % nbuf], 16)
                # Output DMA once DVE finishes the sigmoid tail.
                sync.wait_ge(red_sem, R0 + 6)
                sync.dma_start(out=out_e[:, :], in_=outs[:, :]).then_inc(out_sem, 16)
                sync.wait_ge(out_sem, 16)

            @block.scalar
            def _(scalar):
                # Param loads ride the ACT HWDGE queue so they don't delay
                # the x stream on the sync queue.
                scalar.dma_start(out=w1ts[:, :, :], in_=w1t_e[:, :, :]).then_inc(
                    param_sem, 16
                )
                scalar.dma_start(out=a2s[:, :], in_=a2_e[:, :]).then_inc(param_sem, 16)
                scalar.dma_start(out=w4ts[:, :], in_=w4t_e[:, :]).then_inc(
                    param_sem, 16
                )
                scalar.dma_start(out=scals[:, :], in_=scal_e[:, :]).then_inc(
                    param_sem, 16
                )
                scalar.dma_start(out=eyes[:, :], in_=eye_e[:, :]).then_inc(
                    param_sem, 16
                )
                # Preload the exp table set during the stream; every ACT
                # transcendental below is exp, so no tail table loads.
                c0 = nc.const_aps.tensor(0.0, (1, 1))
                scalar.activation(de1[:, :], c0, Exp)
                # Epilogue: exp(w2*y1) with fused free-dim sum (softmax denom),
                # reading y1 straight out of PSUM.
                scalar.wait_ge(param_sem, 80)
                scalar.wait_ge(pe_sem, 8)
                scalar.activation(
                    es[:, :], y1_ps[:, :], Exp, scale=scals[:, 0:1],
                    accum_out=ss[:, :],
                ).then_inc(act_sem, 1)
                scalar.wait_ge(pe_sem, 10)
                scalar.copy(y3ts[:, :], y3t_ps[:, :]).then_inc(act_sem, 1)
                # sigmoid(z) = 0.5*tanh(z/2) + 0.5; tanh shares the exp
                # table set, so no table load on the critical path.
                scalar.wait_ge(pe_sem, 11)
                scalar.activation(
                    esig[:, :], o_ps[:, :], mybir.ActivationFunctionType.Tanh,
                    scale=0.5,
                ).then_inc(act_sem, 1)

            @block.vector
            def _(vector):
                for i, (b0, nb, ct, hw0, nhw, pidx) in enumerate(jobs):
                    vector.wait_ge(slot_sems[i % nbuf], 16 * (i // nbuf + 1))
                    dst = (
                        yt[:, ct, b0:b0 + nb]
                        if pidx is None
                        else ytx[:, pidx:pidx + 1]
                    )
                    vector.reduce_sum(
                        dst,
                        bufs[i % nbuf][:, 0:nb, 0:nhw],
                        axis=mybir.AxisListType.X,
                    ).then_inc(red_sem, 1)
                # Combine the tapered chunks: yt[:, CT-1, BL-1] = sum(ytx)
                vector.wait_ge(red_sem, njobs)
                vector.reduce_sum(
                    yt[:, CT - 1, BL - 1:BL], ytx[:, :],
                    axis=mybir.AxisListType.X,
                ).then_inc(red_sem, 1)
                # Epilogue.
                vector.wait_ge(pe_sem, 7)
                vector.tensor_copy(y1ts[:, :], y1t_ps[:, :]).then_inc(red_sem, 1)
                vector.wait_ge(act_sem, 1)
                vector.reciprocal(rs[:, :], ss[:, :]).then_inc(red_sem, 1)
                vector.wait_ge(red_sem, R0 + 2)
                # t1 = (es * 1/s) * y1  (y1 read from PSUM)
                vector.scalar_tensor_tensor(
                    t1s[:, :], es[:, :], rs[:, 0:1], y1_ps[:, :],
                    op0=mybir.AluOpType.mult, op1=mybir.AluOpType.mult,
                ).then_inc(red_sem, 1)
                vector.wait_ge(pe_sem, 9)
                vector.wait_ge(red_sem, R0 + 3)
                vector.tensor_add(y2s[:, :], t1s[:, :], p2_ps[:, :]).then_inc(
                    red_sem, 1
                )
                vector.wait_ge(red_sem, R0 + 4)
                vector.tensor_scalar(
                    y3s[:, :],
                    y2s[:, :],
                    scals[:, 1:2],
                    0.0,
                    op0=mybir.AluOpType.mult,
                    op1=mybir.AluOpType.max,
                ).then_inc(red_sem, 1)
                # Final sigmoid tail: outs = 0.5*tanh + 0.5
                vector.wait_ge(act_sem, 3)
                vector.tensor_scalar(
                    outs[:, :], esig[:, :], 0.5, 0.5,
                    op0=mybir.AluOpType.mult, op1=mybir.AluOpType.add,
                ).then_inc(red_sem, 1)

            @block.tensor
            def _(tensor):
                tensor.wait_ge(param_sem, 80)
                # W1 matmuls per channel tile, issued as soon as that tile of
                # yt is fully reduced (overlaps the remaining stream).
                for ct in range(CT):
                    done = R0 if ct == CT - 1 else ct_done[ct]
                    tensor.wait_ge(red_sem, done)
                    tensor.matmul(
                        y1t_ps[:, :],
                        w1ts[:, ct, :],
                        yt[:, ct, :],
                        start=(ct == 0),
                        stop=(ct == CT - 1),
                    ).then_inc(pe_sem, 1)
                    tensor.matmul(
                        y1_ps[:, :],
                        yt[:, ct, :],
                        w1ts[:, ct, :],
                        start=(ct == 0),
                        stop=(ct == CT - 1),
                    ).then_inc(pe_sem, 1)
                # p2[b, k] = sum_h y1T[h, b] * A2[h, k]
                tensor.wait_ge(red_sem, R0 + 1)
                tensor.matmul(
                    p2_ps[:, :], y1ts[:, :], a2s[:, :], start=True, stop=True
                ).then_inc(pe_sem, 1)
                # y3T = transpose(y3)
                tensor.wait_ge(red_sem, R0 + 5)
                tensor.transpose(y3t_ps[:, :], y3s[:, :], eyes[:, :]).then_inc(
                    pe_sem, 1
                )
                # out[b, o] = sum_h y3T[h, b] * W4T[h, o]
                tensor.wait_ge(act_sem, 2)
                tensor.matmul(
                    o_ps[:, :], y3ts[:, :], w4ts[:, :], start=True, stop=True
                ).then_inc(pe_sem, 1)

    return nc


def prep_in_maps(x, W1, A2, w2, w3, W4, hw: int = HW):
    """Shard x over batch; replicate (pre-transposed) params."""
    x = np.ascontiguousarray(np.asarray(x, dtype=np.float32))
    # W1T with the mean scale folded in: [c, h] -> [128, CT, HIDE] with
    # w1t[p, ct, h] = W1[h, ct*128+p] / hw
    w1t = np.ascontiguousarray(
        (np.asarray(W1, np.float32).T / hw).reshape(CT, 128, HIDE).transpose(1, 0, 2)
    )
    import ml_dtypes

    a2 = np.ascontiguousarray(np.asarray(A2, np.float32)).astype(ml_dtypes.bfloat16)
    w4t = np.ascontiguousarray(np.asarray(W4, np.float32).T).astype(
        ml_dtypes.bfloat16
    )
    scal = np.empty((BL, 2), np.float32)
    scal[:, 0] = np.float32(w2)
    scal[:, 1] = np.float32(w3)
    eye8 = np.eye(BL, dtype=np.float32)

    in_maps = []
    for c in range(NCORES):
        xs = x[c * BL:(c + 1) * BL].reshape(BL, CT, 128, hw)
        in_maps.append(
            {
                "x": xs,
                "w1t": w1t,
                "a2": a2,
                "w4t": w4t,
                "scal": scal,
                "eye8": eye8,
            }
        )
    return in_maps


def run(inputs: dict, trace: bool = False, tmpdir: str | None = None):
    """Build + run on 8 cores. Returns (full_output, BassKernelResults)."""
    nc = build_nc()
    in_maps = prep_in_maps(
        inputs["x"], inputs["W1"], inputs["A2"], inputs["w2"], inputs["w3"],
        inputs["W4"],
    )
    res = run_bass_kernel_spmd(
        nc, in_maps, core_ids=list(range(NCORES)), trace=trace, tmpdir=tmpdir
    )
    out = np.concatenate([res.results[c]["out"] for c in range(NCORES)], axis=0)
    return out.reshape(B, OP, 1, 1).astype(np.float32), res


def kernel(**inputs) -> np.ndarray:
    out, _ = run(inputs, trace=False)
    return out
